# revision 86
# baseline (speedup 1.0000x reference)
"""Trainium2 Bass kernel v3: dark-channel + 15x15 erosion, data-parallel
over 8 NeuronCores.

Input  I: [32, 3, 512, 512] f32, k: scalar (15)
Output:   [32, 1, 512, 512] f32

Per core (4 images), all intermediates bf16:
  1. Loads spread across three queues per image: c0 on SP and c1 on ACT
     read the high 16-bit halves of each f32 directly from a bf16 VIEW of
     the input buffer (truncation cast, no Pool involvement); c2 goes
     through the Pool (SWDGE) casting DMA.  The bf16-view DMAs are split
     in w-halves because their 1-element descriptors fold to a >=2^16
     ISA dim otherwise.
  2. Channel min (2 DVE tensor_tensor ops) into a padded filter buffer.
  3. Horizontal 15-min-filter: 4 dyadic shifted mins on DVE (bf16 2x).
  4. PE transpose (identity matmul) -> PSUM, ACT evac per 128-col block.
  5. Vertical 15-min-filter on DVE as a single 4-block chain (fewer DVE
     init overheads than per-pair chains).
  6. Direct transposed store: the bf16 result is DMA'd straight from
     the column layout to HBM (the HBM side is a strided transposed
     view; 2-byte descriptors cost 2x in the model but eliminate the
     back-transposes, the out tile, and all out-evacuations).  Stores
     alternate SP/ACT per block; the host widens to f32 after gather.

The emission is software-pipelined (loads run ~2 images ahead; image
i's H-filter overlaps image i-1's transpose/V/output phase).  The Tile
scheduler turns emission order into per-engine priorities.

HW-legality notes (walrus backend): min-accumulate DMAs are rejected in
any mode, and TensorTensor/TensorScalarPtr are rejected on the Pool
engine - every elementwise min therefore runs on DVE.  DVE is the
bottleneck engine at ~81% occupancy; loads/evacs/stores/transposes hide
behind it on SP/ACT/Pool/PE.

bf16 intermediates: min is selection, not arithmetic; rel err ~3e-3 on
uniform[0,1) data (tolerance 2e-2).  Pad value 30000.0 acts as +inf.

The walrus backend encodes at most ONE sync-wait per instruction; the
post-pass at the end of _build_nc hoists extra waits onto single-wait
NOPs (identical semantics).  CoreSim can't execute the NOPs, so the sim
path builds with split_waits=False.
"""

import sys

if "/opt/trn_rl_repo" not in sys.path:
    sys.path.insert(0, "/opt/trn_rl_repo")

import numpy as np

N_CORES = 8
IMGS = 4          # images per core
C = 3
H = W = 512
K = 15
PAD = K // 2      # 7
L = 8             # left pad in filter buffers (>= PAD+1, power of 2)
PITCH = L + 512 + 8   # 528
NJ = H // 128     # row tiles
NB = W // 128     # col blocks
PADV = 30000.0    # effective +inf for data in [0,1)

_cache = {}

# Dyadic 15-tap min filter: (offset_a, offset_b, lo, hi) per stage, on a
# PITCH-padded buffer with logical x at [L, L+512).  Stage s output f_s
# defined on [lo, hi); final result = f[1:513] min f[8:520] -> [512].
STAGES = [
    (0, 1, 0, 526),
    (0, 2, 0, 524),
    (0, 4, 0, 520),
    (1, 8, 0, 512),   # res[w] = f8[w+1] min f8[w+8]
]


_OFF_H = ()
# All elementwise mins must run on DVE: the walrus HW backend rejects
# TensorTensor/TensorScalarPtr on Pool and min-accumulate DMAs outright.
_EVAC_DVE = ()
_LOAD_PLAN = {i: "mixp" for i in range(IMGS)}
_POOL_V = ()
_MERGE_ENG = {i: ("dve", "dve") for i in range(IMGS)}
_V_SPLIT = 1
_CHAN_ENG = {i: ("sp", "act", "pool") for i in range(IMGS)}
_STORE_ACT = tuple((i, b) for i in range(IMGS) for b in (1, 3))


def _build_nc(split_waits=True, off_h=_OFF_H, off_v=(), work_bufs=4,
              res_bufs=6, out_bufs=2, psum_bufs=8, copy_eng="sync",
              par_load=(), act_warm=True, pl_bufs=3, two_chain=(),
              evac_dve=_EVAC_DVE, h_split=1, v_split=_V_SPLIT,
              out_eng="sync", halves=True, pool_h=(), pool_v=_POOL_V,
              sched="pipe", one_shot=(), h_group={0: 2},
              load_plan=_LOAD_PLAN, v0_psum=False,
              oevac_dve=(), order="LHS",
              chan_eng=_CHAN_ENG, merge_eng=_MERGE_ENG,
              store_act=_STORE_ACT, merge_prio=0, v_block=False,
              v_whole=True, direct_store=True, merge_chain=False, preload=2,
              store3=()):
    """off_h/off_v: iterable of (img, stage) pairs offloaded to DMA.
    pool_h/pool_v: stage indices run on the Pool (gpsimd) engine; either a
    flat iterable (all images) or a dict {img: iterable}.  pool_v entries
    may be ints (both pairs) or (stage, pair) tuples.
    h_group: dict {img: j-tiles per H group} for finer fill pipelining."""
    import concourse.bass as bass
    import concourse.mybir as mybir
    import concourse.tile as tile
    import concourse.masks as masks

    F32 = mybir.dt.float32
    FI = mybir.dt.bfloat16
    MIN = mybir.AluOpType.min

    off_h = set(off_h)
    off_v = set(off_v)
    h_group = dict(h_group or {})
    store_act = set(store_act)
    oevac_dve = set(oevac_dve)

    def _resolve(cfg, i):
        if isinstance(cfg, dict):
            return tuple(cfg.get(i, ()))
        return tuple(cfg)

    def _v_stages(cfg, i, bp):
        out = set()
        for e in _resolve(cfg, i):
            if isinstance(e, tuple):
                if e[1] == bp:
                    out.add(e[0])
            else:
                out.add(e)
        return out

    nc = bass.Bass("TRN2", target_bir_lowering=False, debug=False)

    def pool_min(dst, a, b):
        # Pool-engine elementwise min.  Walrus rejects InstTensorTensor on
        # Pool; the fused scalar_tensor_tensor (a*1.0) min b lowers to an
        # instruction the backend accepts.  *1.0 is exact for bf16.
        nc.gpsimd.scalar_tensor_tensor(
            dst, a, 1.0, b, op0=mybir.AluOpType.mult, op1=MIN)

    inp = nc.dram_tensor("inp", [IMGS, C, H, W], F32, kind="ExternalInput")
    # Same bytes viewed as pairs of bf16 words: [..., 1] is the high half of
    # each f32, i.e. the value truncated to bf16.  Lets SP/ACT HWDGE queues
    # load inputs without the Pool-only casting path.
    inpb = nc.dram_tensor("inpb", [IMGS, C, H, W, 2], FI,
                          kind="ExternalInput")
    # bf16 output in HBM; kernel() widens to f32 on the host after gather.
    out = nc.dram_tensor("out", [IMGS, 1, H, W], FI, kind="ExternalOutput")

    def dyadic(pool, src, n, off=(), split_last=0, pool_stages=(), start=0):
        """15-wide min filter along last dim of src [128, n, PITCH].
        Returns [128, n, 512] f16.  Stages whose index is in `off` run as
        DMA copy (HWDGE) + min-accumulate (Pool SWDGE) instead of DVE.
        Stages in `pool_stages` run on the Pool (gpsimd) engine.
        split_last=q emits the final stage as q chunks along the output
        columns so downstream transposes can start earlier.
        start=k skips the first k stages (src already holds stage k-1)."""
        cur = src
        copy_q = getattr(nc, "sync" if copy_eng == "sync" else "scalar")
        for s, (oa, ob, lo, hi) in list(enumerate(STAGES))[start:]:
            last = s == len(STAGES) - 1
            shape = [128, n, 512 if last else PITCH]
            tag = "res" if last else ("fa" if s % 2 == 0 else "fb")
            nxt = pool.tile(shape, FI, tag=tag, name=f"f{s}")
            use_pool = s in pool_stages
            emin = (pool_min if use_pool else
                    (lambda d, a, b: nc.vector.tensor_tensor(d, a, b, op=MIN)))
            if last and split_last > 1 and s not in off:
                cw = 512 // split_last
                for q in range(split_last):
                    emin(
                        nxt[:, :, q * cw : (q + 1) * cw],
                        cur[:, :, oa + q * cw : oa + (q + 1) * cw],
                        cur[:, :, ob + q * cw : ob + (q + 1) * cw],
                    )
                cur = nxt
                continue
            dst = nxt[:] if last else nxt[:, :, lo:hi]
            if s in off:
                copy_q.dma_start(dst, cur[:, :, oa + lo : oa + hi])
                nc.gpsimd.dma_start(
                    dst, cur[:, :, ob + lo : ob + hi], accum_op=MIN
                )
            else:
                emin(
                    dst, cur[:, :, oa + lo : oa + hi],
                    cur[:, :, ob + lo : ob + hi],
                )
            cur = nxt
        return cur

    with tile.TileContext(nc) as tc:
        with (
            tc.tile_pool(name="const", bufs=1) as cpool,
            tc.tile_pool(name="work", bufs=work_bufs) as work,
            tc.tile_pool(name="resw", bufs=res_bufs) as resw,
            tc.tile_pool(name="opool", bufs=out_bufs) as opool,
            tc.tile_pool(name="plp", bufs=pl_bufs) as plp,
            tc.tile_pool(name="psum", bufs=psum_bufs, space="PSUM") as psum,
        ):
            ident = cpool.tile([128, 128], FI)
            masks.make_identity(nc, ident[:])
            if act_warm:
                # Touch ScalarE once so the activation-table load happens
                # during the DMA fill, not on the first PSUM evac.
                warm = cpool.tile([128, 1], FI)
                nc.scalar.copy(warm[:], ident[:, 0:1])

            def _mode(i):
                if load_plan and i in load_plan:
                    return load_plan[i]
                if i in one_shot:
                    return "os"
                if i in par_load:
                    return "par"
                if i in two_chain:
                    return "ydbl"
                return "chain"

            def emit_load(i):
                # Issue the input DMAs for image i (no compute).  Returns
                # (xpad, aux) where aux holds staging tiles whose channel
                # merge is deferred to emit_H.
                xpad = work.tile([128, NJ, PITCH], FI, tag="xp", name="xpad")
                nc.gpsimd.memset(xpad[:, :, 0:L], PADV)
                nc.gpsimd.memset(xpad[:, :, L + W : PITCH], PADV)
                interior = xpad[:, :, L : L + W]
                src = lambda c: inp[i, c].rearrange("(j p) w -> p j w", p=128)
                # bf16 view of channel c (truncated f32 high halves).  The
                # stride-2 innermost dim forces 1-element descriptors, so a
                # DMA AP can carry at most one j-tile (3 dims incl. the
                # trailing [1,1]); emit one DMA per j.
                srcb = lambda c, j: inpb[i, c].rearrange(
                    "(j p) w t -> p j w t", p=128)[:, j, :, 1]

                def load_b(eng, dst3, c):
                    # dst3: [128, NJ, 512] view.  A full 512-wide stride-2
                    # src folds to a 65536-element ISA dim (16-bit field), so
                    # the per-j mains carry 511 columns (1022 != 1024 breaks
                    # the fold) and one batched DMA fixes up column 511.
                    for j in range(NJ):
                        for hh in range(2):
                            cs = slice(256 * hh, 256 * (hh + 1))
                            eng.dma_start(dst3[:, j, cs], srcb(c, j)[:, cs])

                def load_chan(code, dst3, c):
                    if code == "sp":
                        load_b(nc.sync, dst3, c)
                    elif code == "act":
                        load_b(nc.scalar, dst3, c)
                    elif code in ("sa", "as"):
                        # split: j 0-1 on one HWDGE queue, j 2-3 on the
                        # other ("sa" = SP first, "as" = ACT first)
                        e0, e1 = ((nc.sync, nc.scalar) if code == "sa"
                                  else (nc.scalar, nc.sync))
                        for j in range(NJ):
                            eng = e0 if j < NJ // 2 else e1
                            for hh in range(2):
                                cs = slice(256 * hh, 256 * (hh + 1))
                                eng.dma_start(dst3[:, j, cs],
                                              srcb(c, j)[:, cs])
                    elif code == "poolh":
                        # Pool cast in two j-half DMAs (earlier first half)
                        for hh in range(2):
                            sl = slice(2 * hh, 2 * (hh + 1))
                            nc.gpsimd.dma_start(
                                dst3[:, sl],
                                inp[i, c, 256 * hh : 256 * (hh + 1)]
                                .rearrange("(j p) w -> p j w", p=128))
                    else:
                        nc.gpsimd.dma_start(dst3, src(c))
                m = _mode(i)
                if m == "os":
                    # Single casting DMA for all 3 channels ((c j) folds
                    # contiguously in HBM); channel min on DVE later.
                    abc = plp.tile([128, C, NJ, W], FI, tag="pabc",
                                   name="os_abc")
                    srcall = inp[i].rearrange(
                        "c (j p) w -> p (c j) w", p=128
                    ).rearrange("p (c j) w -> p c j w", c=C)
                    nc.gpsimd.dma_start(abc[:], srcall)
                    return xpad, abc
                if m == "par":
                    # 3 independent casting DMAs; channel min on DVE later.
                    pb = [
                        plp.tile([128, NJ, W], FI, tag=f"pb{c}", name=f"pl{c}")
                        for c in range(C)
                    ]
                    for c in range(C):
                        nc.gpsimd.dma_start(pb[c][:], src(c))
                    return xpad, pb
                if m == "ydbl":
                    # One double-width casting DMA (channels 0+1), then a
                    # min-accumulate DMA folds channel 2 into the first half;
                    # a single DVE merge (deferred) makes the dark channel.
                    ab = plp.tile([128, 2, NJ, W], FI, tag="pb01", name="tc_ab")
                    src01 = inp[i, 0:2].rearrange(
                        "c (j p) w -> p (c j) w", p=128
                    ).rearrange("p (c j) w -> p c j w", c=2)
                    nc.gpsimd.dma_start(ab[:], src01)
                    nc.gpsimd.dma_start(ab[:, 0], src(2), accum_op=MIN)
                    return xpad, ab
                if m == "mixp":
                    # Accumulate-free 3-engine load (HW supports no DMA min):
                    # per-channel engine from chan_eng[i] (default sp/act/
                    # pool); c0 -> interior, c1 -> pbb, c2 -> pbc; the two
                    # merge TT ops are emitted in emit_H.
                    ce = (chan_eng or {}).get(i, ("sp", "act", "pool"))
                    pbb = plp.tile([128, NJ, W], FI, tag="pbb", name="mx_b")
                    pbc = plp.tile([128, NJ, W], FI, tag="pbc", name="mx_c")
                    for c, dst in ((0, interior), (1, pbb[:]), (2, pbc[:])):
                        load_chan(ce[c], dst, c)
                    return xpad, (pbb, pbc)
                # "chain": 3-link min-accumulate chain directly into the
                # padded filter buffer; zero compute-engine work.
                for c in range(C):
                    nc.gpsimd.dma_start(
                        interior, src(c),
                        accum_op=mybir.AluOpType.bypass if c == 0 else MIN,
                    )
                return xpad, None

            def emit_H(i, xpad, aux):
                # Channel merge (if deferred) + horizontal filter, emitted in
                # j-groups of `g` row-tiles for finer pipelining.
                g = h_group.get(i, NJ)
                ph = set(_resolve(pool_h, i))
                m = _mode(i)
                outs = []
                for j0 in range(0, NJ, g):
                    sl = slice(j0, j0 + g)
                    interior = xpad[:, sl, L : L + W]
                    if m == "os":
                        abc = aux
                        nc.vector.tensor_tensor(
                            interior, abc[:, 0, sl], abc[:, 1, sl], op=MIN)
                        nc.vector.tensor_tensor(
                            interior, interior, abc[:, 2, sl], op=MIN)
                    elif m == "par":
                        pb = aux
                        nc.vector.tensor_tensor(
                            interior, pb[0][:, sl], pb[1][:, sl], op=MIN)
                        nc.vector.tensor_tensor(
                            interior, interior, pb[2][:, sl], op=MIN)
                    elif m == "ydbl":
                        ab = aux
                        nc.vector.tensor_tensor(
                            interior, ab[:, 0, sl], ab[:, 1, sl], op=MIN)
                    elif m == "mixp":
                        pbb, pbc = aux
                        me = (merge_eng or {}).get(i, ("dve", "dve"))
                        dvemin = (lambda d, a, b:
                                  nc.vector.tensor_tensor(d, a, b, op=MIN))
                        e1 = pool_min if me[0] == "pool" else dvemin
                        e2 = pool_min if me[1] == "pool" else dvemin
                        import contextlib
                        mp_ctx = (tc.high_priority(offset=-merge_prio)
                                  if merge_prio else contextlib.nullcontext())
                        with mp_ctx:
                            if merge_chain:
                                # chain through interior: c0+c1 first, the
                                # slow Pool-cast c2 last
                                e1(interior, interior, pbb[:, sl])
                                e2(interior, interior, pbc[:, sl])
                            else:
                                # merge pbb,pbc first (independent of c0)
                                e1(pbb[:, sl], pbb[:, sl], pbc[:, sl])
                                e2(interior, interior, pbb[:, sl])
                    outs.append(dyadic(
                        resw, xpad[:, sl, :], g,
                        off={s for (ii, s) in off_h if ii == i},
                        split_last=h_split, pool_stages=ph))
                return g, outs

            def stage2(i, hr):
                # transpose to column layout; the first V stage reads PSUM
                # directly (fused with the evacuation when v0_psum).
                g, routs = hr
                pts = []
                for b in range(NB):
                    pt = psum.tile([128, NJ, 128], FI, tag="pt", name="pt")
                    for j in range(NJ):
                        nc.tensor.transpose(
                            pt[:, j, :],
                            routs[j // g][:, j % g, 128 * b : 128 * (b + 1)],
                            ident[:],
                        )
                    pts.append(pt[:, :, :].rearrange("p n w -> p (n w)"))

                vsp = (v_split.get(i, 2) if isinstance(v_split, dict)
                       else v_split)
                offv = {s for (ii, s) in off_v if ii == i}
                u_pairs = []
                if v0_psum:
                    # Fused evac + V stage 0: f2[x] = min(f[x], f[x+1]) with
                    # the 512 real rows read straight out of PSUM; boundary
                    # columns come from the +inf padding.
                    for bp in range(2):
                        f2 = resw.tile([128, 2, PITCH], FI, tag="fa",
                                       name="v0f2")
                        nc.gpsimd.memset(f2[:, :, 0:L - 1], PADV)
                        nc.gpsimd.memset(f2[:, :, L + H : 526], PADV)
                        for n in range(2):
                            flat = pts[2 * bp + n]
                            eng = nc.vector
                            eng.tensor_tensor(
                                f2[:, n, L : L + H - 1],
                                flat[:, 0 : H - 1], flat[:, 1:H], op=MIN)
                            eng.tensor_copy(f2[:, n, L - 1 : L],
                                            flat[:, 0:1])
                            eng.tensor_copy(f2[:, n, L + H - 1 : L + H],
                                            flat[:, H - 1 : H])
                        u_pairs.append(dyadic(
                            resw, f2, 2, off=offv, split_last=vsp,
                            pool_stages=_v_stages(pool_v, i, bp), start=1))
                else:
                    vb = work.tile([128, NB, PITCH], FI, tag="vb", name="vb")
                    nc.gpsimd.memset(vb[:, :, 0:L], PADV)
                    nc.gpsimd.memset(vb[:, :, L + H : PITCH], PADV)
                    for b in range(NB):
                        if (i, b) in evac_dve:
                            nc.vector.tensor_copy(vb[:, b, L : L + H], pts[b])
                        else:
                            nc.scalar.copy(vb[:, b, L : L + H], pts[b])
                    if v_whole:
                        # One 4-block V chain: half the op count (fewer DVE
                        # init overheads) at the cost of waiting for all
                        # four evacs before stage 0.
                        u4 = dyadic(resw, vb, NB, off=offv, split_last=vsp,
                                    pool_stages=_v_stages(pool_v, i, 0))
                        u_pairs = [u4[:, 0:2, :], u4[:, 2:4, :]]
                    else:
                        u_pairs = [
                            dyadic(resw, vb[:, 2 * bp : 2 * (bp + 1), :], 2,
                                   off=offv, split_last=vsp,
                                   pool_stages=_v_stages(pool_v, i, bp))
                            for bp in range(2)
                        ]

                if direct_store:
                    # Store straight from the column layout: HBM side is a
                    # transposed strided view (partition = column).  Skips
                    # the back-transposes, the o tile, and the out-evacs.
                    def blk_dst(b, r0=0, r1=H):
                        return out[i, 0, r0:r1,
                                   128 * b : 128 * (b + 1)].rearrange(
                                       "r c -> c r")

                    if i in store3:
                        # drain-critical image: 3 store queues (SP/ACT/Pool)
                        # with block 3 split in row-halves so the longest
                        # engine chain is 790+500 instead of 790+790
                        nc.sync.dma_start(blk_dst(0), u_pairs[0][:, 0, :])
                        nc.scalar.dma_start(blk_dst(1), u_pairs[0][:, 1, :])
                        nc.gpsimd.dma_start(blk_dst(2), u_pairs[1][:, 0, :])
                        hh = H // 2
                        nc.sync.dma_start(blk_dst(3, 0, hh),
                                          u_pairs[1][:, 1, 0:hh])
                        nc.scalar.dma_start(blk_dst(3, hh, H),
                                            u_pairs[1][:, 1, hh:H])
                        return
                    for b in range(NB):
                        st_eng = (nc.scalar if (i, b) in store_act
                                  else nc.sync)
                        st_eng.dma_start(blk_dst(b),
                                         u_pairs[b // 2][:, b % 2, :])
                    return

                # transpose back, f16 out, store per row-tile
                o = opool.tile([128, NJ, W], FI, name="o")
                for j in range(NJ):
                    pt = psum.tile([128, NB, 128], FI, tag="pt", name="pt2")
                    for b in range(NB):
                        nc.tensor.transpose(
                            pt[:, b, :],
                            u_pairs[b // 2][:, b % 2, 128 * j : 128 * (j + 1)],
                            ident[:],
                        )
                    pt_f = pt[:, :, :].rearrange("p n w -> p (n w)")
                    if (i, j) in oevac_dve:
                        nc.vector.tensor_copy(o[:, j, :], pt_f)
                    else:
                        nc.scalar.copy(o[:, j, :], pt_f)
                    st_eng = (nc.scalar if (i, j) in store_act else nc.sync)
                    st_eng.dma_start(
                        out[i, 0, 128 * j : 128 * (j + 1)].rearrange(
                            "(q p) w -> p q w", p=128
                        ),
                        o[:, j : j + 1, :],
                    )

            if sched == "pipe":
                # Software pipeline: loads run ~2 images ahead; image i's
                # H-filter overlaps image i-1's transpose/V/output phase.
                loads = {}
                hres = {}
                for i in range(min(preload, IMGS)):
                    loads[i] = emit_load(i)
                hres[0] = emit_H(0, *loads[0])
                for i in range(1, IMGS):
                    if order == "LSH":
                        if i + preload - 1 < IMGS and i + preload - 1 not in loads:
                            loads[i + preload - 1] = emit_load(i + preload - 1)
                        stage2(i - 1, hres[i - 1])
                        hres[i] = emit_H(i, *loads[i])
                    elif order == "HSL":
                        hres[i] = emit_H(i, *loads[i])
                        stage2(i - 1, hres[i - 1])
                        if i + preload - 1 < IMGS and i + preload - 1 not in loads:
                            loads[i + preload - 1] = emit_load(i + preload - 1)
                    else:
                        if i + preload - 1 < IMGS and i + preload - 1 not in loads:
                            loads[i + preload - 1] = emit_load(i + preload - 1)
                        hres[i] = emit_H(i, *loads[i])
                        stage2(i - 1, hres[i - 1])
                stage2(IMGS - 1, hres[IMGS - 1])
            else:
                for i in range(IMGS):
                    stage2(i, emit_H(i, *emit_load(i)))

    # bass emits accumulate DMAs as mode="Copy"; the walrus birverifier
    # requires mode="CCE" for cce_op != bypass.
    for bb in nc.main_func.blocks:
        for ins in bb.instructions:
            if (type(ins).__name__ == "InstDMACopy"
                    and getattr(ins, "cce_op", None) is not None
                    and str(ins.cce_op) == "AluOpType.min"):
                ins.mode = "CCE"

    if not split_waits:
        return nc
    import concourse.mybir as mybir
    nsplit = 0
    for bb in nc.main_func.blocks:
        idx = 0
        while idx < len(bb.instructions):
            ins = bb.instructions[idx]
            si = ins.sync_info
            if si is not None and si.on_wait and len(si.on_wait) > 1:
                waits = list(si.on_wait)
                for w in waits[:-1]:
                    nop = mybir.InstNoOp(
                        name=f"W-split-{nsplit}", ins=[], outs=[]
                    )
                    nop.engine = ins.engine
                    nop.sync_info = mybir.SyncInfo(on_wait=[w], on_update=[])
                    bb.instructions.insert(idx, nop)
                    nsplit += 1
                    idx += 1
                ins.sync_info = mybir.SyncInfo(
                    on_wait=[waits[-1]], on_update=list(si.on_update or [])
                )
            idx += 1
    return nc


def _get_nc():
    if "nc" not in _cache:
        _cache["nc"] = _build_nc()
    return _cache["nc"]


def kernel(I, k):
    from concourse.bass_utils import run_bass_kernel_spmd

    k = int(np.asarray(k))
    assert k == K, f"kernel compiled for k={K}, got {k}"
    I = np.ascontiguousarray(np.asarray(I), dtype=np.float32)
    B = I.shape[0]
    assert I.shape == (B, C, H, W) and B == N_CORES * IMGS

    nc = _get_nc()
    import ml_dtypes

    Ib = I.view(ml_dtypes.bfloat16).reshape(B, C, H, W, 2)
    in_maps = [
        {
            "inp": I[c * IMGS : (c + 1) * IMGS],
            "inpb": Ib[c * IMGS : (c + 1) * IMGS],
        }
        for c in range(N_CORES)
    ]
    res = run_bass_kernel_spmd(nc, in_maps, list(range(N_CORES))).results
    return np.concatenate(
        [np.asarray(res[c]["out"]).astype(np.float32) for c in range(N_CORES)],
        axis=0,
    )



# revision 88
# speedup vs baseline: 1.0120x; 1.0120x over previous
"""Trainium2 Bass kernel v3: dark-channel + 15x15 erosion, data-parallel
over 8 NeuronCores.

Input  I: [32, 3, 512, 512] f32, k: scalar (15)
Output:   [32, 1, 512, 512] f32

Per core (4 images), all intermediates bf16:
  1. Loads spread across three queues per image: c0 on SP and c1 on ACT
     read the high 16-bit halves of each f32 directly from a bf16 VIEW of
     the input buffer (truncation cast, no Pool involvement); c2 goes
     through the Pool (SWDGE) casting DMA.  The bf16-view DMAs are split
     in w-halves because their 1-element descriptors fold to a >=2^16
     ISA dim otherwise.
  2. Channel min (2 DVE tensor_tensor ops) into a padded filter buffer.
  3. Horizontal 15-min-filter: 4 dyadic shifted mins on DVE (bf16 2x).
  4. PE transpose (identity matmul) -> PSUM, ACT evac per 128-col block.
  5. Vertical 15-min-filter on DVE as a single 4-block chain (fewer DVE
     init overheads than per-pair chains).
  6. Direct transposed store: the bf16 result is DMA'd straight from
     the column layout to HBM (the HBM side is a strided transposed
     view; 2-byte descriptors cost 2x in the model but eliminate the
     back-transposes, the out tile, and all out-evacuations).  Stores
     alternate SP/ACT per block; the host widens to f32 after gather.

The emission is software-pipelined (loads run ~2 images ahead; image
i's H-filter overlaps image i-1's transpose/V/output phase).  The Tile
scheduler turns emission order into per-engine priorities.

HW-legality notes (walrus backend): min-accumulate DMAs are rejected in
any mode, and TensorTensor/TensorScalarPtr are rejected on the Pool
engine - every elementwise min therefore runs on DVE.  DVE is the
bottleneck engine at ~81% occupancy; loads/evacs/stores/transposes hide
behind it on SP/ACT/Pool/PE.

bf16 intermediates: min is selection, not arithmetic; rel err ~3e-3 on
uniform[0,1) data (tolerance 2e-2).  Pad value 30000.0 acts as +inf.

The walrus backend encodes at most ONE sync-wait per instruction; the
post-pass at the end of _build_nc hoists extra waits onto single-wait
NOPs (identical semantics).  CoreSim can't execute the NOPs, so the sim
path builds with split_waits=False.
"""

import sys

if "/opt/trn_rl_repo" not in sys.path:
    sys.path.insert(0, "/opt/trn_rl_repo")

import numpy as np

N_CORES = 8
IMGS = 4          # images per core
C = 3
H = W = 512
K = 15
PAD = K // 2      # 7
L = 8             # left pad in filter buffers (>= PAD+1, power of 2)
PITCH = L + 512 + 8   # 528
NJ = H // 128     # row tiles
NB = W // 128     # col blocks
PADV = 30000.0    # effective +inf for data in [0,1)

_cache = {}

# Dyadic 15-tap min filter: (offset_a, offset_b, lo, hi) per stage, on a
# PITCH-padded buffer with logical x at [L, L+512).  Stage s output f_s
# defined on [lo, hi); final result = f[1:513] min f[8:520] -> [512].
STAGES = [
    (0, 1, 0, 526),
    (0, 2, 0, 524),
    (0, 4, 0, 520),
    (1, 8, 0, 512),   # res[w] = f8[w+1] min f8[w+8]
]


_OFF_H = ()
# All elementwise mins must run on DVE: the walrus HW backend rejects
# TensorTensor/TensorScalarPtr on Pool and min-accumulate DMAs outright.
_EVAC_DVE = ()
_LOAD_PLAN = {i: "mixp" for i in range(IMGS)}
_POOL_V = ()
_MERGE_ENG = {i: ("dve", "dve") for i in range(IMGS)}
_V_SPLIT = 1
_CHAN_ENG = {i: ("sp", "act", "pool") for i in range(IMGS)}
_CHAN_ENG[0] = ("sp", "act", "poolq")
_STORE_ACT = tuple((i, b) for i in range(IMGS) for b in (1, 3))


def _build_nc(split_waits=True, off_h=_OFF_H, off_v=(), work_bufs=4,
              res_bufs=6, out_bufs=2, psum_bufs=8, copy_eng="sync",
              par_load=(), act_warm=True, pl_bufs=3, two_chain=(),
              evac_dve=_EVAC_DVE, h_split=1, v_split=_V_SPLIT,
              out_eng="sync", halves=True, pool_h=(), pool_v=_POOL_V,
              sched="pipe", one_shot=(), h_group={0: (1, 1, 2)},
              load_plan=_LOAD_PLAN, v0_psum=False,
              oevac_dve=(), order="LHS",
              chan_eng=_CHAN_ENG, merge_eng=_MERGE_ENG,
              store_act=_STORE_ACT, merge_prio=0, v_block=False,
              v_whole=True, direct_store=True, merge_chain=False, preload=2,
              store3=()):
    """off_h/off_v: iterable of (img, stage) pairs offloaded to DMA.
    pool_h/pool_v: stage indices run on the Pool (gpsimd) engine; either a
    flat iterable (all images) or a dict {img: iterable}.  pool_v entries
    may be ints (both pairs) or (stage, pair) tuples.
    h_group: dict {img: j-tiles per H group} for finer fill pipelining."""
    import concourse.bass as bass
    import concourse.mybir as mybir
    import concourse.tile as tile
    import concourse.masks as masks

    F32 = mybir.dt.float32
    FI = mybir.dt.bfloat16
    MIN = mybir.AluOpType.min

    off_h = set(off_h)
    off_v = set(off_v)
    h_group = dict(h_group or {})
    store_act = set(store_act)
    oevac_dve = set(oevac_dve)

    def _resolve(cfg, i):
        if isinstance(cfg, dict):
            return tuple(cfg.get(i, ()))
        return tuple(cfg)

    def _v_stages(cfg, i, bp):
        out = set()
        for e in _resolve(cfg, i):
            if isinstance(e, tuple):
                if e[1] == bp:
                    out.add(e[0])
            else:
                out.add(e)
        return out

    nc = bass.Bass("TRN2", target_bir_lowering=False, debug=False)

    def pool_min(dst, a, b):
        # Pool-engine elementwise min.  Walrus rejects InstTensorTensor on
        # Pool; the fused scalar_tensor_tensor (a*1.0) min b lowers to an
        # instruction the backend accepts.  *1.0 is exact for bf16.
        nc.gpsimd.scalar_tensor_tensor(
            dst, a, 1.0, b, op0=mybir.AluOpType.mult, op1=MIN)

    inp = nc.dram_tensor("inp", [IMGS, C, H, W], F32, kind="ExternalInput")
    # Same bytes viewed as pairs of bf16 words: [..., 1] is the high half of
    # each f32, i.e. the value truncated to bf16.  Lets SP/ACT HWDGE queues
    # load inputs without the Pool-only casting path.
    inpb = nc.dram_tensor("inpb", [IMGS, C, H, W, 2], FI,
                          kind="ExternalInput")
    # bf16 output in HBM; kernel() widens to f32 on the host after gather.
    out = nc.dram_tensor("out", [IMGS, 1, H, W], FI, kind="ExternalOutput")

    def dyadic(pool, src, n, off=(), split_last=0, pool_stages=(), start=0):
        """15-wide min filter along last dim of src [128, n, PITCH].
        Returns [128, n, 512] f16.  Stages whose index is in `off` run as
        DMA copy (HWDGE) + min-accumulate (Pool SWDGE) instead of DVE.
        Stages in `pool_stages` run on the Pool (gpsimd) engine.
        split_last=q emits the final stage as q chunks along the output
        columns so downstream transposes can start earlier.
        start=k skips the first k stages (src already holds stage k-1)."""
        cur = src
        copy_q = getattr(nc, "sync" if copy_eng == "sync" else "scalar")
        for s, (oa, ob, lo, hi) in list(enumerate(STAGES))[start:]:
            last = s == len(STAGES) - 1
            shape = [128, n, 512 if last else PITCH]
            tag = "res" if last else ("fa" if s % 2 == 0 else "fb")
            nxt = pool.tile(shape, FI, tag=tag, name=f"f{s}")
            use_pool = s in pool_stages
            emin = (pool_min if use_pool else
                    (lambda d, a, b: nc.vector.tensor_tensor(d, a, b, op=MIN)))
            if last and split_last > 1 and s not in off:
                cw = 512 // split_last
                for q in range(split_last):
                    emin(
                        nxt[:, :, q * cw : (q + 1) * cw],
                        cur[:, :, oa + q * cw : oa + (q + 1) * cw],
                        cur[:, :, ob + q * cw : ob + (q + 1) * cw],
                    )
                cur = nxt
                continue
            dst = nxt[:] if last else nxt[:, :, lo:hi]
            if s in off:
                copy_q.dma_start(dst, cur[:, :, oa + lo : oa + hi])
                nc.gpsimd.dma_start(
                    dst, cur[:, :, ob + lo : ob + hi], accum_op=MIN
                )
            else:
                emin(
                    dst, cur[:, :, oa + lo : oa + hi],
                    cur[:, :, ob + lo : ob + hi],
                )
            cur = nxt
        return cur

    with tile.TileContext(nc) as tc:
        with (
            tc.tile_pool(name="const", bufs=1) as cpool,
            tc.tile_pool(name="work", bufs=work_bufs) as work,
            tc.tile_pool(name="resw", bufs=res_bufs) as resw,
            tc.tile_pool(name="opool", bufs=out_bufs) as opool,
            tc.tile_pool(name="plp", bufs=pl_bufs) as plp,
            tc.tile_pool(name="psum", bufs=psum_bufs, space="PSUM") as psum,
        ):
            ident = cpool.tile([128, 128], FI)
            masks.make_identity(nc, ident[:])
            if act_warm:
                # Touch ScalarE once so the activation-table load happens
                # during the DMA fill, not on the first PSUM evac.
                warm = cpool.tile([128, 1], FI)
                nc.scalar.copy(warm[:], ident[:, 0:1])

            def _mode(i):
                if load_plan and i in load_plan:
                    return load_plan[i]
                if i in one_shot:
                    return "os"
                if i in par_load:
                    return "par"
                if i in two_chain:
                    return "ydbl"
                return "chain"

            def emit_load(i):
                # Issue the input DMAs for image i (no compute).  Returns
                # (xpad, aux) where aux holds staging tiles whose channel
                # merge is deferred to emit_H.
                xpad = work.tile([128, NJ, PITCH], FI, tag="xp", name="xpad")
                nc.gpsimd.memset(xpad[:, :, 0:L], PADV)
                nc.gpsimd.memset(xpad[:, :, L + W : PITCH], PADV)
                interior = xpad[:, :, L : L + W]
                src = lambda c: inp[i, c].rearrange("(j p) w -> p j w", p=128)
                # bf16 view of channel c (truncated f32 high halves).  The
                # stride-2 innermost dim forces 1-element descriptors, so a
                # DMA AP can carry at most one j-tile (3 dims incl. the
                # trailing [1,1]); emit one DMA per j.
                srcb = lambda c, j: inpb[i, c].rearrange(
                    "(j p) w t -> p j w t", p=128)[:, j, :, 1]

                def load_b(eng, dst3, c):
                    # dst3: [128, NJ, 512] view.  A full 512-wide stride-2
                    # src folds to a 65536-element ISA dim (16-bit field), so
                    # the per-j mains carry 511 columns (1022 != 1024 breaks
                    # the fold) and one batched DMA fixes up column 511.
                    for j in range(NJ):
                        for hh in range(2):
                            cs = slice(256 * hh, 256 * (hh + 1))
                            eng.dma_start(dst3[:, j, cs], srcb(c, j)[:, cs])

                def load_chan(code, dst3, c):
                    if code == "sp":
                        load_b(nc.sync, dst3, c)
                    elif code == "act":
                        load_b(nc.scalar, dst3, c)
                    elif code in ("sa", "as"):
                        # split: j 0-1 on one HWDGE queue, j 2-3 on the
                        # other ("sa" = SP first, "as" = ACT first)
                        e0, e1 = ((nc.sync, nc.scalar) if code == "sa"
                                  else (nc.scalar, nc.sync))
                        for j in range(NJ):
                            eng = e0 if j < NJ // 2 else e1
                            for hh in range(2):
                                cs = slice(256 * hh, 256 * (hh + 1))
                                eng.dma_start(dst3[:, j, cs],
                                              srcb(c, j)[:, cs])
                    elif code == "poolq":
                        for j in range(NJ):
                            nc.gpsimd.dma_start(
                                dst3[:, j : j + 1],
                                inp[i, c, 128 * j : 128 * (j + 1)]
                                .rearrange("(q p) w -> p q w", p=128))
                    elif code == "poolh":
                        # Pool cast in two j-half DMAs (earlier first half)
                        for hh in range(2):
                            sl = slice(2 * hh, 2 * (hh + 1))
                            nc.gpsimd.dma_start(
                                dst3[:, sl],
                                inp[i, c, 256 * hh : 256 * (hh + 1)]
                                .rearrange("(j p) w -> p j w", p=128))
                    else:
                        nc.gpsimd.dma_start(dst3, src(c))
                m = _mode(i)
                if m == "os":
                    # Single casting DMA for all 3 channels ((c j) folds
                    # contiguously in HBM); channel min on DVE later.
                    abc = plp.tile([128, C, NJ, W], FI, tag="pabc",
                                   name="os_abc")
                    srcall = inp[i].rearrange(
                        "c (j p) w -> p (c j) w", p=128
                    ).rearrange("p (c j) w -> p c j w", c=C)
                    nc.gpsimd.dma_start(abc[:], srcall)
                    return xpad, abc
                if m == "par":
                    # 3 independent casting DMAs; channel min on DVE later.
                    pb = [
                        plp.tile([128, NJ, W], FI, tag=f"pb{c}", name=f"pl{c}")
                        for c in range(C)
                    ]
                    for c in range(C):
                        nc.gpsimd.dma_start(pb[c][:], src(c))
                    return xpad, pb
                if m == "ydbl":
                    # One double-width casting DMA (channels 0+1), then a
                    # min-accumulate DMA folds channel 2 into the first half;
                    # a single DVE merge (deferred) makes the dark channel.
                    ab = plp.tile([128, 2, NJ, W], FI, tag="pb01", name="tc_ab")
                    src01 = inp[i, 0:2].rearrange(
                        "c (j p) w -> p (c j) w", p=128
                    ).rearrange("p (c j) w -> p c j w", c=2)
                    nc.gpsimd.dma_start(ab[:], src01)
                    nc.gpsimd.dma_start(ab[:, 0], src(2), accum_op=MIN)
                    return xpad, ab
                if m == "mixp":
                    # Accumulate-free 3-engine load (HW supports no DMA min):
                    # per-channel engine from chan_eng[i] (default sp/act/
                    # pool); c0 -> interior, c1 -> pbb, c2 -> pbc; the two
                    # merge TT ops are emitted in emit_H.
                    ce = (chan_eng or {}).get(i, ("sp", "act", "pool"))
                    pbb = plp.tile([128, NJ, W], FI, tag="pbb", name="mx_b")
                    pbc = plp.tile([128, NJ, W], FI, tag="pbc", name="mx_c")
                    for c, dst in ((0, interior), (1, pbb[:]), (2, pbc[:])):
                        load_chan(ce[c], dst, c)
                    return xpad, (pbb, pbc)
                # "chain": 3-link min-accumulate chain directly into the
                # padded filter buffer; zero compute-engine work.
                for c in range(C):
                    nc.gpsimd.dma_start(
                        interior, src(c),
                        accum_op=mybir.AluOpType.bypass if c == 0 else MIN,
                    )
                return xpad, None

            def emit_H(i, xpad, aux):
                # Channel merge (if deferred) + horizontal filter, emitted in
                # j-groups of `g` row-tiles for finer pipelining.
                gv = h_group.get(i, NJ)
                sizes = list(gv) if isinstance(gv, tuple) else None
                ph = set(_resolve(pool_h, i))
                m = _mode(i)
                outs = []
                bounds = []
                if sizes:
                    a = 0
                    for s_ in sizes:
                        bounds.append((a, a + s_))
                        a += s_
                else:
                    bounds = [(j0, j0 + gv) for j0 in range(0, NJ, gv)]
                for (j0, j1) in bounds:
                    g = j1 - j0
                    sl = slice(j0, j1)
                    interior = xpad[:, sl, L : L + W]
                    if m == "os":
                        abc = aux
                        nc.vector.tensor_tensor(
                            interior, abc[:, 0, sl], abc[:, 1, sl], op=MIN)
                        nc.vector.tensor_tensor(
                            interior, interior, abc[:, 2, sl], op=MIN)
                    elif m == "par":
                        pb = aux
                        nc.vector.tensor_tensor(
                            interior, pb[0][:, sl], pb[1][:, sl], op=MIN)
                        nc.vector.tensor_tensor(
                            interior, interior, pb[2][:, sl], op=MIN)
                    elif m == "ydbl":
                        ab = aux
                        nc.vector.tensor_tensor(
                            interior, ab[:, 0, sl], ab[:, 1, sl], op=MIN)
                    elif m == "mixp":
                        pbb, pbc = aux
                        me = (merge_eng or {}).get(i, ("dve", "dve"))
                        dvemin = (lambda d, a, b:
                                  nc.vector.tensor_tensor(d, a, b, op=MIN))
                        e1 = pool_min if me[0] == "pool" else dvemin
                        e2 = pool_min if me[1] == "pool" else dvemin
                        import contextlib
                        mp_ctx = (tc.high_priority(offset=-merge_prio)
                                  if merge_prio else contextlib.nullcontext())
                        with mp_ctx:
                            if merge_chain:
                                # chain through interior: c0+c1 first, the
                                # slow Pool-cast c2 last
                                e1(interior, interior, pbb[:, sl])
                                e2(interior, interior, pbc[:, sl])
                            else:
                                # merge pbb,pbc first (independent of c0)
                                e1(pbb[:, sl], pbb[:, sl], pbc[:, sl])
                                e2(interior, interior, pbb[:, sl])
                    outs.append(dyadic(
                        resw, xpad[:, sl, :], g,
                        off={s for (ii, s) in off_h if ii == i},
                        split_last=h_split, pool_stages=ph))
                gmap = {}
                for gi, (j0, j1) in enumerate(bounds):
                    for j in range(j0, j1):
                        gmap[j] = (gi, j - j0)
                return gmap, outs

            def stage2(i, hr):
                # transpose to column layout; the first V stage reads PSUM
                # directly (fused with the evacuation when v0_psum).
                gmap, routs = hr
                pts = []
                for b in range(NB):
                    pt = psum.tile([128, NJ, 128], FI, tag="pt", name="pt")
                    for j in range(NJ):
                        nc.tensor.transpose(
                            pt[:, j, :],
                            routs[gmap[j][0]][:, gmap[j][1],
                                              128 * b : 128 * (b + 1)],
                            ident[:],
                        )
                    pts.append(pt[:, :, :].rearrange("p n w -> p (n w)"))

                vsp = (v_split.get(i, 2) if isinstance(v_split, dict)
                       else v_split)
                offv = {s for (ii, s) in off_v if ii == i}
                u_pairs = []
                if v0_psum:
                    # Fused evac + V stage 0: f2[x] = min(f[x], f[x+1]) with
                    # the 512 real rows read straight out of PSUM; boundary
                    # columns come from the +inf padding.
                    for bp in range(2):
                        f2 = resw.tile([128, 2, PITCH], FI, tag="fa",
                                       name="v0f2")
                        nc.gpsimd.memset(f2[:, :, 0:L - 1], PADV)
                        nc.gpsimd.memset(f2[:, :, L + H : 526], PADV)
                        for n in range(2):
                            flat = pts[2 * bp + n]
                            eng = nc.vector
                            eng.tensor_tensor(
                                f2[:, n, L : L + H - 1],
                                flat[:, 0 : H - 1], flat[:, 1:H], op=MIN)
                            eng.tensor_copy(f2[:, n, L - 1 : L],
                                            flat[:, 0:1])
                            eng.tensor_copy(f2[:, n, L + H - 1 : L + H],
                                            flat[:, H - 1 : H])
                        u_pairs.append(dyadic(
                            resw, f2, 2, off=offv, split_last=vsp,
                            pool_stages=_v_stages(pool_v, i, bp), start=1))
                else:
                    vb = work.tile([128, NB, PITCH], FI, tag="vb", name="vb")
                    nc.gpsimd.memset(vb[:, :, 0:L], PADV)
                    nc.gpsimd.memset(vb[:, :, L + H : PITCH], PADV)
                    for b in range(NB):
                        if (i, b) in evac_dve:
                            nc.vector.tensor_copy(vb[:, b, L : L + H], pts[b])
                        else:
                            nc.scalar.copy(vb[:, b, L : L + H], pts[b])
                    if v_whole:
                        # One 4-block V chain: half the op count (fewer DVE
                        # init overheads) at the cost of waiting for all
                        # four evacs before stage 0.
                        u4 = dyadic(resw, vb, NB, off=offv, split_last=vsp,
                                    pool_stages=_v_stages(pool_v, i, 0))
                        u_pairs = [u4[:, 0:2, :], u4[:, 2:4, :]]
                    else:
                        u_pairs = [
                            dyadic(resw, vb[:, 2 * bp : 2 * (bp + 1), :], 2,
                                   off=offv, split_last=vsp,
                                   pool_stages=_v_stages(pool_v, i, bp))
                            for bp in range(2)
                        ]

                if direct_store:
                    # Store straight from the column layout: HBM side is a
                    # transposed strided view (partition = column).  Skips
                    # the back-transposes, the o tile, and the out-evacs.
                    def blk_dst(b, r0=0, r1=H):
                        return out[i, 0, r0:r1,
                                   128 * b : 128 * (b + 1)].rearrange(
                                       "r c -> c r")

                    if i in store3:
                        # drain-critical image: 3 store queues (SP/ACT/Pool)
                        # with block 3 split in row-halves so the longest
                        # engine chain is 790+500 instead of 790+790
                        nc.sync.dma_start(blk_dst(0), u_pairs[0][:, 0, :])
                        nc.scalar.dma_start(blk_dst(1), u_pairs[0][:, 1, :])
                        nc.gpsimd.dma_start(blk_dst(2), u_pairs[1][:, 0, :])
                        hh = H // 2
                        nc.sync.dma_start(blk_dst(3, 0, hh),
                                          u_pairs[1][:, 1, 0:hh])
                        nc.scalar.dma_start(blk_dst(3, hh, H),
                                            u_pairs[1][:, 1, hh:H])
                        return
                    for b in range(NB):
                        st_eng = (nc.scalar if (i, b) in store_act
                                  else nc.sync)
                        st_eng.dma_start(blk_dst(b),
                                         u_pairs[b // 2][:, b % 2, :])
                    return

                # transpose back, f16 out, store per row-tile
                o = opool.tile([128, NJ, W], FI, name="o")
                for j in range(NJ):
                    pt = psum.tile([128, NB, 128], FI, tag="pt", name="pt2")
                    for b in range(NB):
                        nc.tensor.transpose(
                            pt[:, b, :],
                            u_pairs[b // 2][:, b % 2, 128 * j : 128 * (j + 1)],
                            ident[:],
                        )
                    pt_f = pt[:, :, :].rearrange("p n w -> p (n w)")
                    if (i, j) in oevac_dve:
                        nc.vector.tensor_copy(o[:, j, :], pt_f)
                    else:
                        nc.scalar.copy(o[:, j, :], pt_f)
                    st_eng = (nc.scalar if (i, j) in store_act else nc.sync)
                    st_eng.dma_start(
                        out[i, 0, 128 * j : 128 * (j + 1)].rearrange(
                            "(q p) w -> p q w", p=128
                        ),
                        o[:, j : j + 1, :],
                    )

            if sched == "pipe":
                # Software pipeline: loads run ~2 images ahead; image i's
                # H-filter overlaps image i-1's transpose/V/output phase.
                loads = {}
                hres = {}
                for i in range(min(preload, IMGS)):
                    loads[i] = emit_load(i)
                hres[0] = emit_H(0, *loads[0])
                for i in range(1, IMGS):
                    if order == "LSH":
                        if i + preload - 1 < IMGS and i + preload - 1 not in loads:
                            loads[i + preload - 1] = emit_load(i + preload - 1)
                        stage2(i - 1, hres[i - 1])
                        hres[i] = emit_H(i, *loads[i])
                    elif order == "HSL":
                        hres[i] = emit_H(i, *loads[i])
                        stage2(i - 1, hres[i - 1])
                        if i + preload - 1 < IMGS and i + preload - 1 not in loads:
                            loads[i + preload - 1] = emit_load(i + preload - 1)
                    else:
                        if i + preload - 1 < IMGS and i + preload - 1 not in loads:
                            loads[i + preload - 1] = emit_load(i + preload - 1)
                        hres[i] = emit_H(i, *loads[i])
                        stage2(i - 1, hres[i - 1])
                stage2(IMGS - 1, hres[IMGS - 1])
            else:
                for i in range(IMGS):
                    stage2(i, emit_H(i, *emit_load(i)))

    # bass emits accumulate DMAs as mode="Copy"; the walrus birverifier
    # requires mode="CCE" for cce_op != bypass.
    for bb in nc.main_func.blocks:
        for ins in bb.instructions:
            if (type(ins).__name__ == "InstDMACopy"
                    and getattr(ins, "cce_op", None) is not None
                    and str(ins.cce_op) == "AluOpType.min"):
                ins.mode = "CCE"

    if not split_waits:
        return nc
    import concourse.mybir as mybir
    nsplit = 0
    for bb in nc.main_func.blocks:
        idx = 0
        while idx < len(bb.instructions):
            ins = bb.instructions[idx]
            si = ins.sync_info
            if si is not None and si.on_wait and len(si.on_wait) > 1:
                waits = list(si.on_wait)
                for w in waits[:-1]:
                    nop = mybir.InstNoOp(
                        name=f"W-split-{nsplit}", ins=[], outs=[]
                    )
                    nop.engine = ins.engine
                    nop.sync_info = mybir.SyncInfo(on_wait=[w], on_update=[])
                    bb.instructions.insert(idx, nop)
                    nsplit += 1
                    idx += 1
                ins.sync_info = mybir.SyncInfo(
                    on_wait=[waits[-1]], on_update=list(si.on_update or [])
                )
            idx += 1
    return nc


def _get_nc():
    if "nc" not in _cache:
        _cache["nc"] = _build_nc()
    return _cache["nc"]


def kernel(I, k):
    from concourse.bass_utils import run_bass_kernel_spmd

    k = int(np.asarray(k))
    assert k == K, f"kernel compiled for k={K}, got {k}"
    I = np.ascontiguousarray(np.asarray(I), dtype=np.float32)
    B = I.shape[0]
    assert I.shape == (B, C, H, W) and B == N_CORES * IMGS

    nc = _get_nc()
    import ml_dtypes

    Ib = I.view(ml_dtypes.bfloat16).reshape(B, C, H, W, 2)
    in_maps = [
        {
            "inp": I[c * IMGS : (c + 1) * IMGS],
            "inpb": Ib[c * IMGS : (c + 1) * IMGS],
        }
        for c in range(N_CORES)
    ]
    res = run_bass_kernel_spmd(nc, in_maps, list(range(N_CORES))).results
    return np.concatenate(
        [np.asarray(res[c]["out"]).astype(np.float32) for c in range(N_CORES)],
        axis=0,
    )



# revision 90
# speedup vs baseline: 1.0222x; 1.0101x over previous
"""Trainium2 Bass kernel v3: dark-channel + 15x15 erosion, data-parallel
over 8 NeuronCores.

Input  I: [32, 3, 512, 512] f32, k: scalar (15)
Output:   [32, 1, 512, 512] f32

Per core (4 images), all intermediates bf16:
  1. Loads spread across three queues per image: c0 on SP and c1 on ACT
     read the high 16-bit halves of each f32 directly from a bf16 VIEW of
     the input buffer (truncation cast, no Pool involvement); c2 goes
     through the Pool (SWDGE) casting DMA.  The bf16-view DMAs are split
     in w-halves because their 1-element descriptors fold to a >=2^16
     ISA dim otherwise.
  2. Channel min (2 DVE tensor_tensor ops) into a padded filter buffer.
  3. Horizontal 15-min-filter: 4 dyadic shifted mins on DVE (bf16 2x).
  4. PE transpose (identity matmul) -> PSUM, ACT evac per 128-col block.
  5. Vertical 15-min-filter on DVE as a single 4-block chain (fewer DVE
     init overheads than per-pair chains).
  6. Direct transposed store: the bf16 result is DMA'd straight from
     the column layout to HBM (the HBM side is a strided transposed
     view; 2-byte descriptors cost 2x in the model but eliminate the
     back-transposes, the out tile, and all out-evacuations).  Stores
     alternate SP/ACT per block; the host widens to f32 after gather.

The emission is software-pipelined (loads run ~2 images ahead; image
i's H-filter overlaps image i-1's transpose/V/output phase).  The Tile
scheduler turns emission order into per-engine priorities.

HW-legality notes (walrus backend): min-accumulate DMAs are rejected in
any mode, and TensorTensor/TensorScalarPtr are rejected on the Pool
engine - every elementwise min therefore runs on DVE.  DVE is the
bottleneck engine at ~81% occupancy; loads/evacs/stores/transposes hide
behind it on SP/ACT/Pool/PE.

bf16 intermediates: min is selection, not arithmetic; rel err ~3e-3 on
uniform[0,1) data (tolerance 2e-2).  Pad value 30000.0 acts as +inf.

The walrus backend encodes at most ONE sync-wait per instruction; the
post-pass at the end of _build_nc hoists extra waits onto single-wait
NOPs (identical semantics).  CoreSim can't execute the NOPs, so the sim
path builds with split_waits=False.
"""

import sys

if "/opt/trn_rl_repo" not in sys.path:
    sys.path.insert(0, "/opt/trn_rl_repo")

import numpy as np

N_CORES = 8
IMGS = 4          # images per core
C = 3
H = W = 512
K = 15
PAD = K // 2      # 7
L = 8             # left pad in filter buffers (>= PAD+1, power of 2)
PITCH = L + 512 + 8   # 528
NJ = H // 128     # row tiles
NB = W // 128     # col blocks
PADV = 30000.0    # effective +inf for data in [0,1)

_cache = {}

# Dyadic 15-tap min filter: (offset_a, offset_b, lo, hi) per stage, on a
# PITCH-padded buffer with logical x at [L, L+512).  Stage s output f_s
# defined on [lo, hi); final result = f[1:513] min f[8:520] -> [512].
STAGES = [
    (0, 1, 0, 526),
    (0, 2, 0, 524),
    (0, 4, 0, 520),
    (1, 8, 0, 512),   # res[w] = f8[w+1] min f8[w+8]
]


_OFF_H = ()
# All elementwise mins must run on DVE: the walrus HW backend rejects
# TensorTensor/TensorScalarPtr on Pool and min-accumulate DMAs outright.
_EVAC_DVE = ()
_LOAD_PLAN = {i: "mixp" for i in range(IMGS)}
_POOL_V = ()
_MERGE_ENG = {i: ("dve", "dve") for i in range(IMGS)}
_V_SPLIT = 1
_CHAN_ENG = {i: ("sp", "act", "pool") for i in range(IMGS)}
_CHAN_ENG[0] = ("sp", "act", "poolq")
_STORE_ACT = tuple((i, b) for i in range(IMGS) for b in (1, 3))


def _build_nc(split_waits=True, off_h=_OFF_H, off_v=(), work_bufs=4,
              res_bufs=6, out_bufs=2, psum_bufs=8, copy_eng="sync",
              par_load=(), act_warm=True, pl_bufs=3, two_chain=(),
              evac_dve=_EVAC_DVE, h_split=1, v_split=_V_SPLIT,
              out_eng="sync", halves=True, pool_h=(), pool_v=_POOL_V,
              sched="pipe", one_shot=(), h_group={0: (1, 1, 2)},
              load_plan=_LOAD_PLAN, v0_psum=False,
              oevac_dve=(), order="LHS",
              chan_eng=_CHAN_ENG, merge_eng=_MERGE_ENG,
              store_act=_STORE_ACT, merge_prio=0, v_block=False,
              v_whole=True, direct_store=True, merge_chain=False, preload=2,
              store3=(), vnsplit=(3,)):
    """off_h/off_v: iterable of (img, stage) pairs offloaded to DMA.
    pool_h/pool_v: stage indices run on the Pool (gpsimd) engine; either a
    flat iterable (all images) or a dict {img: iterable}.  pool_v entries
    may be ints (both pairs) or (stage, pair) tuples.
    h_group: dict {img: j-tiles per H group} for finer fill pipelining."""
    import concourse.bass as bass
    import concourse.mybir as mybir
    import concourse.tile as tile
    import concourse.masks as masks

    F32 = mybir.dt.float32
    FI = mybir.dt.bfloat16
    MIN = mybir.AluOpType.min

    off_h = set(off_h)
    off_v = set(off_v)
    h_group = dict(h_group or {})
    store_act = set(store_act)
    oevac_dve = set(oevac_dve)

    def _resolve(cfg, i):
        if isinstance(cfg, dict):
            return tuple(cfg.get(i, ()))
        return tuple(cfg)

    def _v_stages(cfg, i, bp):
        out = set()
        for e in _resolve(cfg, i):
            if isinstance(e, tuple):
                if e[1] == bp:
                    out.add(e[0])
            else:
                out.add(e)
        return out

    nc = bass.Bass("TRN2", target_bir_lowering=False, debug=False)

    def pool_min(dst, a, b):
        # Pool-engine elementwise min.  Walrus rejects InstTensorTensor on
        # Pool; the fused scalar_tensor_tensor (a*1.0) min b lowers to an
        # instruction the backend accepts.  *1.0 is exact for bf16.
        nc.gpsimd.scalar_tensor_tensor(
            dst, a, 1.0, b, op0=mybir.AluOpType.mult, op1=MIN)

    inp = nc.dram_tensor("inp", [IMGS, C, H, W], F32, kind="ExternalInput")
    # Same bytes viewed as pairs of bf16 words: [..., 1] is the high half of
    # each f32, i.e. the value truncated to bf16.  Lets SP/ACT HWDGE queues
    # load inputs without the Pool-only casting path.
    inpb = nc.dram_tensor("inpb", [IMGS, C, H, W, 2], FI,
                          kind="ExternalInput")
    # bf16 output in HBM; kernel() widens to f32 on the host after gather.
    out = nc.dram_tensor("out", [IMGS, 1, H, W], FI, kind="ExternalOutput")

    def dyadic(pool, src, n, off=(), split_last=0, pool_stages=(), start=0,
               nsplit_final=False):
        """15-wide min filter along last dim of src [128, n, PITCH].
        Returns [128, n, 512] f16.  Stages whose index is in `off` run as
        DMA copy (HWDGE) + min-accumulate (Pool SWDGE) instead of DVE.
        Stages in `pool_stages` run on the Pool (gpsimd) engine.
        split_last=q emits the final stage as q chunks along the output
        columns so downstream transposes can start earlier.
        start=k skips the first k stages (src already holds stage k-1)."""
        cur = src
        copy_q = getattr(nc, "sync" if copy_eng == "sync" else "scalar")
        for s, (oa, ob, lo, hi) in list(enumerate(STAGES))[start:]:
            last = s == len(STAGES) - 1
            shape = [128, n, 512 if last else PITCH]
            tag = "res" if last else ("fa" if s % 2 == 0 else "fb")
            nxt = pool.tile(shape, FI, tag=tag, name=f"f{s}")
            use_pool = s in pool_stages
            emin = (pool_min if use_pool else
                    (lambda d, a, b: nc.vector.tensor_tensor(d, a, b, op=MIN)))
            if last and nsplit_final and n > 1 and s not in off:
                hn = n // 2
                emin(nxt[:, 0:hn, :], cur[:, 0:hn, oa : oa + 512],
                     cur[:, 0:hn, ob : ob + 512])
                emin(nxt[:, hn:n, :], cur[:, hn:n, oa : oa + 512],
                     cur[:, hn:n, ob : ob + 512])
                cur = nxt
                continue
            if last and split_last > 1 and s not in off:
                cw = 512 // split_last
                for q in range(split_last):
                    emin(
                        nxt[:, :, q * cw : (q + 1) * cw],
                        cur[:, :, oa + q * cw : oa + (q + 1) * cw],
                        cur[:, :, ob + q * cw : ob + (q + 1) * cw],
                    )
                cur = nxt
                continue
            dst = nxt[:] if last else nxt[:, :, lo:hi]
            if s in off:
                copy_q.dma_start(dst, cur[:, :, oa + lo : oa + hi])
                nc.gpsimd.dma_start(
                    dst, cur[:, :, ob + lo : ob + hi], accum_op=MIN
                )
            else:
                emin(
                    dst, cur[:, :, oa + lo : oa + hi],
                    cur[:, :, ob + lo : ob + hi],
                )
            cur = nxt
        return cur

    with tile.TileContext(nc) as tc:
        with (
            tc.tile_pool(name="const", bufs=1) as cpool,
            tc.tile_pool(name="work", bufs=work_bufs) as work,
            tc.tile_pool(name="resw", bufs=res_bufs) as resw,
            tc.tile_pool(name="opool", bufs=out_bufs) as opool,
            tc.tile_pool(name="plp", bufs=pl_bufs) as plp,
            tc.tile_pool(name="psum", bufs=psum_bufs, space="PSUM") as psum,
        ):
            ident = cpool.tile([128, 128], FI)
            masks.make_identity(nc, ident[:])
            if act_warm:
                # Touch ScalarE once so the activation-table load happens
                # during the DMA fill, not on the first PSUM evac.
                warm = cpool.tile([128, 1], FI)
                nc.scalar.copy(warm[:], ident[:, 0:1])

            def _mode(i):
                if load_plan and i in load_plan:
                    return load_plan[i]
                if i in one_shot:
                    return "os"
                if i in par_load:
                    return "par"
                if i in two_chain:
                    return "ydbl"
                return "chain"

            def emit_load(i):
                # Issue the input DMAs for image i (no compute).  Returns
                # (xpad, aux) where aux holds staging tiles whose channel
                # merge is deferred to emit_H.
                xpad = work.tile([128, NJ, PITCH], FI, tag="xp", name="xpad")
                nc.gpsimd.memset(xpad[:, :, 0:L], PADV)
                nc.gpsimd.memset(xpad[:, :, L + W : PITCH], PADV)
                interior = xpad[:, :, L : L + W]
                src = lambda c: inp[i, c].rearrange("(j p) w -> p j w", p=128)
                # bf16 view of channel c (truncated f32 high halves).  The
                # stride-2 innermost dim forces 1-element descriptors, so a
                # DMA AP can carry at most one j-tile (3 dims incl. the
                # trailing [1,1]); emit one DMA per j.
                srcb = lambda c, j: inpb[i, c].rearrange(
                    "(j p) w t -> p j w t", p=128)[:, j, :, 1]

                def load_b(eng, dst3, c):
                    # dst3: [128, NJ, 512] view.  A full 512-wide stride-2
                    # src folds to a 65536-element ISA dim (16-bit field), so
                    # the per-j mains carry 511 columns (1022 != 1024 breaks
                    # the fold) and one batched DMA fixes up column 511.
                    for j in range(NJ):
                        for hh in range(2):
                            cs = slice(256 * hh, 256 * (hh + 1))
                            eng.dma_start(dst3[:, j, cs], srcb(c, j)[:, cs])

                def load_chan(code, dst3, c):
                    if code == "sp":
                        load_b(nc.sync, dst3, c)
                    elif code == "act":
                        load_b(nc.scalar, dst3, c)
                    elif code in ("sa", "as"):
                        # split: j 0-1 on one HWDGE queue, j 2-3 on the
                        # other ("sa" = SP first, "as" = ACT first)
                        e0, e1 = ((nc.sync, nc.scalar) if code == "sa"
                                  else (nc.scalar, nc.sync))
                        for j in range(NJ):
                            eng = e0 if j < NJ // 2 else e1
                            for hh in range(2):
                                cs = slice(256 * hh, 256 * (hh + 1))
                                eng.dma_start(dst3[:, j, cs],
                                              srcb(c, j)[:, cs])
                    elif code == "poolq":
                        for j in range(NJ):
                            nc.gpsimd.dma_start(
                                dst3[:, j : j + 1],
                                inp[i, c, 128 * j : 128 * (j + 1)]
                                .rearrange("(q p) w -> p q w", p=128))
                    elif code == "poolh":
                        # Pool cast in two j-half DMAs (earlier first half)
                        for hh in range(2):
                            sl = slice(2 * hh, 2 * (hh + 1))
                            nc.gpsimd.dma_start(
                                dst3[:, sl],
                                inp[i, c, 256 * hh : 256 * (hh + 1)]
                                .rearrange("(j p) w -> p j w", p=128))
                    else:
                        nc.gpsimd.dma_start(dst3, src(c))
                m = _mode(i)
                if m == "os":
                    # Single casting DMA for all 3 channels ((c j) folds
                    # contiguously in HBM); channel min on DVE later.
                    abc = plp.tile([128, C, NJ, W], FI, tag="pabc",
                                   name="os_abc")
                    srcall = inp[i].rearrange(
                        "c (j p) w -> p (c j) w", p=128
                    ).rearrange("p (c j) w -> p c j w", c=C)
                    nc.gpsimd.dma_start(abc[:], srcall)
                    return xpad, abc
                if m == "par":
                    # 3 independent casting DMAs; channel min on DVE later.
                    pb = [
                        plp.tile([128, NJ, W], FI, tag=f"pb{c}", name=f"pl{c}")
                        for c in range(C)
                    ]
                    for c in range(C):
                        nc.gpsimd.dma_start(pb[c][:], src(c))
                    return xpad, pb
                if m == "ydbl":
                    # One double-width casting DMA (channels 0+1), then a
                    # min-accumulate DMA folds channel 2 into the first half;
                    # a single DVE merge (deferred) makes the dark channel.
                    ab = plp.tile([128, 2, NJ, W], FI, tag="pb01", name="tc_ab")
                    src01 = inp[i, 0:2].rearrange(
                        "c (j p) w -> p (c j) w", p=128
                    ).rearrange("p (c j) w -> p c j w", c=2)
                    nc.gpsimd.dma_start(ab[:], src01)
                    nc.gpsimd.dma_start(ab[:, 0], src(2), accum_op=MIN)
                    return xpad, ab
                if m == "mixp":
                    # Accumulate-free 3-engine load (HW supports no DMA min):
                    # per-channel engine from chan_eng[i] (default sp/act/
                    # pool); c0 -> interior, c1 -> pbb, c2 -> pbc; the two
                    # merge TT ops are emitted in emit_H.
                    ce = (chan_eng or {}).get(i, ("sp", "act", "pool"))
                    pbb = plp.tile([128, NJ, W], FI, tag="pbb", name="mx_b")
                    pbc = plp.tile([128, NJ, W], FI, tag="pbc", name="mx_c")
                    for c, dst in ((0, interior), (1, pbb[:]), (2, pbc[:])):
                        load_chan(ce[c], dst, c)
                    return xpad, (pbb, pbc)
                # "chain": 3-link min-accumulate chain directly into the
                # padded filter buffer; zero compute-engine work.
                for c in range(C):
                    nc.gpsimd.dma_start(
                        interior, src(c),
                        accum_op=mybir.AluOpType.bypass if c == 0 else MIN,
                    )
                return xpad, None

            def emit_H(i, xpad, aux):
                # Channel merge (if deferred) + horizontal filter, emitted in
                # j-groups of `g` row-tiles for finer pipelining.
                gv = h_group.get(i, NJ)
                sizes = list(gv) if isinstance(gv, tuple) else None
                ph = set(_resolve(pool_h, i))
                m = _mode(i)
                outs = []
                bounds = []
                if sizes:
                    a = 0
                    for s_ in sizes:
                        bounds.append((a, a + s_))
                        a += s_
                else:
                    bounds = [(j0, j0 + gv) for j0 in range(0, NJ, gv)]
                for (j0, j1) in bounds:
                    g = j1 - j0
                    sl = slice(j0, j1)
                    interior = xpad[:, sl, L : L + W]
                    if m == "os":
                        abc = aux
                        nc.vector.tensor_tensor(
                            interior, abc[:, 0, sl], abc[:, 1, sl], op=MIN)
                        nc.vector.tensor_tensor(
                            interior, interior, abc[:, 2, sl], op=MIN)
                    elif m == "par":
                        pb = aux
                        nc.vector.tensor_tensor(
                            interior, pb[0][:, sl], pb[1][:, sl], op=MIN)
                        nc.vector.tensor_tensor(
                            interior, interior, pb[2][:, sl], op=MIN)
                    elif m == "ydbl":
                        ab = aux
                        nc.vector.tensor_tensor(
                            interior, ab[:, 0, sl], ab[:, 1, sl], op=MIN)
                    elif m == "mixp":
                        pbb, pbc = aux
                        me = (merge_eng or {}).get(i, ("dve", "dve"))
                        dvemin = (lambda d, a, b:
                                  nc.vector.tensor_tensor(d, a, b, op=MIN))
                        e1 = pool_min if me[0] == "pool" else dvemin
                        e2 = pool_min if me[1] == "pool" else dvemin
                        import contextlib
                        mp_ctx = (tc.high_priority(offset=-merge_prio)
                                  if merge_prio else contextlib.nullcontext())
                        with mp_ctx:
                            if merge_chain:
                                # chain through interior: c0+c1 first, the
                                # slow Pool-cast c2 last
                                e1(interior, interior, pbb[:, sl])
                                e2(interior, interior, pbc[:, sl])
                            else:
                                # merge pbb,pbc first (independent of c0)
                                e1(pbb[:, sl], pbb[:, sl], pbc[:, sl])
                                e2(interior, interior, pbb[:, sl])
                    outs.append(dyadic(
                        resw, xpad[:, sl, :], g,
                        off={s for (ii, s) in off_h if ii == i},
                        split_last=h_split, pool_stages=ph))
                gmap = {}
                for gi, (j0, j1) in enumerate(bounds):
                    for j in range(j0, j1):
                        gmap[j] = (gi, j - j0)
                return gmap, outs

            def stage2(i, hr):
                # transpose to column layout; the first V stage reads PSUM
                # directly (fused with the evacuation when v0_psum).
                gmap, routs = hr
                pts = []
                for b in range(NB):
                    pt = psum.tile([128, NJ, 128], FI, tag="pt", name="pt")
                    for j in range(NJ):
                        nc.tensor.transpose(
                            pt[:, j, :],
                            routs[gmap[j][0]][:, gmap[j][1],
                                              128 * b : 128 * (b + 1)],
                            ident[:],
                        )
                    pts.append(pt[:, :, :].rearrange("p n w -> p (n w)"))

                vsp = (v_split.get(i, 2) if isinstance(v_split, dict)
                       else v_split)
                offv = {s for (ii, s) in off_v if ii == i}
                u_pairs = []
                if v0_psum:
                    # Fused evac + V stage 0: f2[x] = min(f[x], f[x+1]) with
                    # the 512 real rows read straight out of PSUM; boundary
                    # columns come from the +inf padding.
                    for bp in range(2):
                        f2 = resw.tile([128, 2, PITCH], FI, tag="fa",
                                       name="v0f2")
                        nc.gpsimd.memset(f2[:, :, 0:L - 1], PADV)
                        nc.gpsimd.memset(f2[:, :, L + H : 526], PADV)
                        for n in range(2):
                            flat = pts[2 * bp + n]
                            eng = nc.vector
                            eng.tensor_tensor(
                                f2[:, n, L : L + H - 1],
                                flat[:, 0 : H - 1], flat[:, 1:H], op=MIN)
                            eng.tensor_copy(f2[:, n, L - 1 : L],
                                            flat[:, 0:1])
                            eng.tensor_copy(f2[:, n, L + H - 1 : L + H],
                                            flat[:, H - 1 : H])
                        u_pairs.append(dyadic(
                            resw, f2, 2, off=offv, split_last=vsp,
                            pool_stages=_v_stages(pool_v, i, bp), start=1))
                else:
                    vb = work.tile([128, NB, PITCH], FI, tag="vb", name="vb")
                    nc.gpsimd.memset(vb[:, :, 0:L], PADV)
                    nc.gpsimd.memset(vb[:, :, L + H : PITCH], PADV)
                    for b in range(NB):
                        if (i, b) in evac_dve:
                            nc.vector.tensor_copy(vb[:, b, L : L + H], pts[b])
                        else:
                            nc.scalar.copy(vb[:, b, L : L + H], pts[b])
                    if v_whole:
                        # One 4-block V chain: half the op count (fewer DVE
                        # init overheads) at the cost of waiting for all
                        # four evacs before stage 0.
                        u4 = dyadic(resw, vb, NB, off=offv, split_last=vsp,
                                    pool_stages=_v_stages(pool_v, i, 0),
                                    nsplit_final=(i in vnsplit))
                        u_pairs = [u4[:, 0:2, :], u4[:, 2:4, :]]
                    else:
                        u_pairs = [
                            dyadic(resw, vb[:, 2 * bp : 2 * (bp + 1), :], 2,
                                   off=offv, split_last=vsp,
                                   pool_stages=_v_stages(pool_v, i, bp))
                            for bp in range(2)
                        ]

                if direct_store:
                    # Store straight from the column layout: HBM side is a
                    # transposed strided view (partition = column).  Skips
                    # the back-transposes, the o tile, and the out-evacs.
                    def blk_dst(b, r0=0, r1=H):
                        return out[i, 0, r0:r1,
                                   128 * b : 128 * (b + 1)].rearrange(
                                       "r c -> c r")

                    if i in store3:
                        # drain-critical image: 3 store queues (SP/ACT/Pool)
                        # with block 3 split in row-halves so the longest
                        # engine chain is 790+500 instead of 790+790
                        nc.sync.dma_start(blk_dst(0), u_pairs[0][:, 0, :])
                        nc.scalar.dma_start(blk_dst(1), u_pairs[0][:, 1, :])
                        nc.gpsimd.dma_start(blk_dst(2), u_pairs[1][:, 0, :])
                        hh = H // 2
                        nc.sync.dma_start(blk_dst(3, 0, hh),
                                          u_pairs[1][:, 1, 0:hh])
                        nc.scalar.dma_start(blk_dst(3, hh, H),
                                            u_pairs[1][:, 1, hh:H])
                        return
                    for b in range(NB):
                        st_eng = (nc.scalar if (i, b) in store_act
                                  else nc.sync)
                        st_eng.dma_start(blk_dst(b),
                                         u_pairs[b // 2][:, b % 2, :])
                    return

                # transpose back, f16 out, store per row-tile
                o = opool.tile([128, NJ, W], FI, name="o")
                for j in range(NJ):
                    pt = psum.tile([128, NB, 128], FI, tag="pt", name="pt2")
                    for b in range(NB):
                        nc.tensor.transpose(
                            pt[:, b, :],
                            u_pairs[b // 2][:, b % 2, 128 * j : 128 * (j + 1)],
                            ident[:],
                        )
                    pt_f = pt[:, :, :].rearrange("p n w -> p (n w)")
                    if (i, j) in oevac_dve:
                        nc.vector.tensor_copy(o[:, j, :], pt_f)
                    else:
                        nc.scalar.copy(o[:, j, :], pt_f)
                    st_eng = (nc.scalar if (i, j) in store_act else nc.sync)
                    st_eng.dma_start(
                        out[i, 0, 128 * j : 128 * (j + 1)].rearrange(
                            "(q p) w -> p q w", p=128
                        ),
                        o[:, j : j + 1, :],
                    )

            if sched == "pipe":
                # Software pipeline: loads run ~2 images ahead; image i's
                # H-filter overlaps image i-1's transpose/V/output phase.
                loads = {}
                hres = {}
                for i in range(min(preload, IMGS)):
                    loads[i] = emit_load(i)
                hres[0] = emit_H(0, *loads[0])
                for i in range(1, IMGS):
                    if order == "LSH":
                        if i + preload - 1 < IMGS and i + preload - 1 not in loads:
                            loads[i + preload - 1] = emit_load(i + preload - 1)
                        stage2(i - 1, hres[i - 1])
                        hres[i] = emit_H(i, *loads[i])
                    elif order == "HSL":
                        hres[i] = emit_H(i, *loads[i])
                        stage2(i - 1, hres[i - 1])
                        if i + preload - 1 < IMGS and i + preload - 1 not in loads:
                            loads[i + preload - 1] = emit_load(i + preload - 1)
                    else:
                        if i + preload - 1 < IMGS and i + preload - 1 not in loads:
                            loads[i + preload - 1] = emit_load(i + preload - 1)
                        hres[i] = emit_H(i, *loads[i])
                        stage2(i - 1, hres[i - 1])
                stage2(IMGS - 1, hres[IMGS - 1])
            else:
                for i in range(IMGS):
                    stage2(i, emit_H(i, *emit_load(i)))

    # bass emits accumulate DMAs as mode="Copy"; the walrus birverifier
    # requires mode="CCE" for cce_op != bypass.
    for bb in nc.main_func.blocks:
        for ins in bb.instructions:
            if (type(ins).__name__ == "InstDMACopy"
                    and getattr(ins, "cce_op", None) is not None
                    and str(ins.cce_op) == "AluOpType.min"):
                ins.mode = "CCE"

    if not split_waits:
        return nc
    import concourse.mybir as mybir
    nsplit = 0
    for bb in nc.main_func.blocks:
        idx = 0
        while idx < len(bb.instructions):
            ins = bb.instructions[idx]
            si = ins.sync_info
            if si is not None and si.on_wait and len(si.on_wait) > 1:
                waits = list(si.on_wait)
                for w in waits[:-1]:
                    nop = mybir.InstNoOp(
                        name=f"W-split-{nsplit}", ins=[], outs=[]
                    )
                    nop.engine = ins.engine
                    nop.sync_info = mybir.SyncInfo(on_wait=[w], on_update=[])
                    bb.instructions.insert(idx, nop)
                    nsplit += 1
                    idx += 1
                ins.sync_info = mybir.SyncInfo(
                    on_wait=[waits[-1]], on_update=list(si.on_update or [])
                )
            idx += 1
    return nc


def _get_nc():
    if "nc" not in _cache:
        _cache["nc"] = _build_nc()
    return _cache["nc"]


def kernel(I, k):
    from concourse.bass_utils import run_bass_kernel_spmd

    k = int(np.asarray(k))
    assert k == K, f"kernel compiled for k={K}, got {k}"
    I = np.ascontiguousarray(np.asarray(I), dtype=np.float32)
    B = I.shape[0]
    assert I.shape == (B, C, H, W) and B == N_CORES * IMGS

    nc = _get_nc()
    import ml_dtypes

    Ib = I.view(ml_dtypes.bfloat16).reshape(B, C, H, W, 2)
    in_maps = [
        {
            "inp": I[c * IMGS : (c + 1) * IMGS],
            "inpb": Ib[c * IMGS : (c + 1) * IMGS],
        }
        for c in range(N_CORES)
    ]
    res = run_bass_kernel_spmd(nc, in_maps, list(range(N_CORES))).results
    return np.concatenate(
        [np.asarray(res[c]["out"]).astype(np.float32) for c in range(N_CORES)],
        axis=0,
    )

